# revision 8
# baseline (speedup 1.0000x reference)
"""Multi-head attention (B=4, N=2048, C=1024, H=16, D=64) on 8 TRN2 NeuronCores.

Sharding: core c owns (batch b = c//2, sequence half = c%2) -> 1024 query
tokens, all 16 heads.  Each core computes K and V for its OWN half only;
the partner half arrives via pairwise AllGathers (replica groups [2b, 2b+1]).
Output is purely row-sharded -> host gather is a concat.

Performance notes (v5):
- TRN2's activity-monitor firmware clamps the PE clock to 50% when PE
  activity stays near 100% for more than ~15-30 3.4us windows, and the clamp
  can persist for hundreds of us.  The QKV front is therefore PACED to ~65%
  activity (the attention phase's natural level, measured sustainable):
  each projection group's first matmul is gated on a small Vector-engine
  pace op chained behind the previous group's bias adds.  The front's wall
  time is bounded by the AllGather chain anyway, so pacing costs nothing.
- The four AllGathers are ordered by when their output is consumed
  (K chunk0, V chunk0, V chunk1, K chunk1) and attention iterates m-tiles
  grouped by V chunk, so no dependency is ever on the critical path.
- exp() is split between the Scalar engine (native Exp) and the Vector
  engine (Schraudolph bit-trick: bf16 is the top half of f32, so
  p = bitcast_bf16(int16(S*scale*184.665 + 16250.5)) is one tensor_scalar).
  Scores PSUM + exp are processed in 512-column halves (4 single-bank PSUM
  slots) so the PV matmuls wait on half-tiles, not full tiles.
- Each head's softmax normalization is deferred into the next head's
  iteration stream so the Vector queue never delays an exp.
- All matmuls bf16 with f32 PSUM accumulate.
"""

import numpy as np
import ml_dtypes

import concourse.bass as bass
import concourse.mybir as mybir
import concourse.tile as tile
from concourse import bacc
from concourse.bass import _add_dep_helper
from concourse.bass_utils import run_bass_kernel_spmd

B, N, C = 4, 2048, 1024
H, D = 16, 64
SCALE = D ** -0.5
NCORES = 8
NQ = N // 2          # query tokens per core (own half)
M = N                # key/value tokens after gather

BF16 = mybir.dt.bfloat16
F32 = mybir.dt.float32
I16 = mybir.dt.int16

# Schraudolph exp in bf16: exp(x*SCALE) ~= bitcast_bf16(int16(x*EXPA + EXPB))
EXPA = (2.0 ** 7 / np.log(2.0)) * SCALE
EXPB = 127.0 * 128.0 - 5.5
# which of the 16 m-tile iterations per head run exp on DVE instead of Scalar
DVE_EXP_IDX = {3, 7, 11, 14}
# m-tiles grouped by V gather chunk (j=mt%8: j<4 -> chunk0, j>=4 -> chunk1)
MT_ORDER = [0, 1, 2, 3, 8, 9, 10, 11, 4, 5, 6, 7, 12, 13, 14, 15]
PACE_N = 700         # pace-op length (f32 elems) -> ~0.5us on DVE

_CACHE = {}
LAST_RESULTS = None


def _build():
    nc = bacc.Bacc(
        "TRN2",
        target_bir_lowering=False,
        debug=False,
        enable_asserts=False,
        num_devices=NCORES,
    )
    xoT = nc.dram_tensor("xoT", [C, NQ], BF16, kind="ExternalInput")
    wqkvT = nc.dram_tensor("wqkvT", [C, 3 * C], BF16, kind="ExternalInput")
    bqk = nc.dram_tensor("bqk", [128, 16], F32, kind="ExternalInput")
    bv = nc.dram_tensor("bv", [1, C], BF16, kind="ExternalInput")
    wprojT = nc.dram_tensor("wprojT", [C, C], BF16, kind="ExternalInput")
    bproj = nc.dram_tensor("bproj", [128, 8], F32, kind="ExternalInput")
    yT = nc.dram_tensor("yT", [C, NQ], F32, kind="ExternalOutput")

    groups = [[2 * b, 2 * b + 1] for b in range(B)]

    with tile.TileContext(nc) as tc:
        with (
            tc.tile_pool(name="persist", bufs=1) as pp,
            tc.tile_pool(name="psum", bufs=1, space="PSUM") as psp,
            tc.tile_pool(name="dram", bufs=1, space="DRAM") as dp,
        ):
            lp = tc.alloc_tile_pool(name="front", bufs=1)

            # ---- inputs; wk on the scalar queue so K can start earliest ----
            wk = lp.tile([128, 8, C], BF16, tag="wk", name="wk")
            nc.scalar.dma_start(
                wk[:, :, :],
                wqkvT.rearrange("(c p) o -> p c o", p=128)[:, :, C : 2 * C],
            )
            xo = lp.tile([128, 8, NQ], BF16, tag="xo", name="xo")
            nc.sync.dma_start(xo[:, :, :], xoT.rearrange("(c p) n -> p c n", p=128))
            wv = lp.tile([128, 8, C], BF16, tag="wv", name="wv")
            nc.sync.dma_start(
                wv[:, :, :],
                wqkvT.rearrange("(c p) o -> p c o", p=128)[:, :, 2 * C : 3 * C],
            )
            wq = lp.tile([128, 8, C], BF16, tag="wq", name="wq")
            nc.sync.dma_start(
                wq[:, :, :],
                wqkvT.rearrange("(c p) o -> p c o", p=128)[:, :, 0:C],
            )

            bqk_sb = pp.tile([128, 16], F32, tag="bqk", name="bqk")
            nc.scalar.dma_start(bqk_sb[:, :], bqk[:, :])
            bv_sb = lp.tile([1, C], BF16, tag="bv", name="bv")
            nc.scalar.dma_start(bv_sb[:, :], bv[:, :])
            bp_sb = pp.tile([128, 8], F32, tag="bp", name="bp")
            nc.scalar.dma_start(bp_sb[:, :], bproj[:, :])

            bvb = lp.tile([128, C], BF16, tag="bvb", name="bvb")
            nc.gpsimd.partition_broadcast(bvb[:, :], bv_sb[:, :])

            # ---- persistent attention operands ----
            KT = pp.tile([128, 8, M], BF16, tag="KT", name="KT")
            QT = pp.tile([128, 8, NQ], BF16, tag="QT", name="QT")
            Vb = [
                pp.tile([128, 2, 4, H, D + 1], BF16, tag=f"Vb{c}", name=f"Vb{c}")
                for c in range(2)
            ]
            A_sb = [
                pp.tile([128, NQ], BF16, tag=f"a{i}", name=f"a{i}") for i in range(8)
            ]

            # staging SBUF + DRAM bounce buffers
            kh = lp.tile([128, 8, NQ], BF16, tag="kh", name="kh")
            vh = lp.tile([128, 8, H, D + 1], BF16, tag="vh", name="vh")
            k_in = [dp.tile([512, NQ], BF16, tag=f"ki{c}", name=f"ki{c}") for c in range(2)]
            k_out = [
                dp.tile([2, 512, NQ], BF16, tag=f"ko{c}", name=f"ko{c}") for c in range(2)
            ]
            v_in = [
                dp.tile([512, H * (D + 1)], BF16, tag=f"vi{c}", name=f"vi{c}")
                for c in range(2)
            ]
            v_out = [
                dp.tile([2, 512, H * (D + 1)], BF16, tag=f"vo{c}", name=f"vo{c}")
                for c in range(2)
            ]

            # ---- HAM pacing machinery ----
            pace_sb = lp.tile([1, 2 * PACE_N], F32, tag="pace", name="pace")
            nc.vector.memset(pace_sb[:, :], 0.0)
            pace_state = {"last": None, "flip": 0}

            def pace_group(first_mm, last_dve):
                # gate this group's first matmul on the previous group's pace
                # op; chain a new pace op behind this group's bias adds.
                if pace_state["last"] is not None:
                    _add_dep_helper(
                        first_mm.ins, pace_state["last"].ins, sync=True,
                        reason="HAM activity pacing",
                    )
                f = pace_state["flip"]
                pace_state["flip"] = 1 - f
                pace_state["last"] = nc.vector.tensor_copy(
                    pace_sb[:, f * PACE_N : (f + 1) * PACE_N],
                    pace_sb[:, (1 - f) * PACE_N : (2 - f) * PACE_N],
                )

            def k_heads(c, which):
                # K/Q output channels i*128..(i+1)*128 for own tokens; bias
                # fused into the PSUM->SBUF copy (split in halves for pacing).
                w_sb, boff, dst = (
                    (wk, 8, kh) if which == "k" else (wq, 0, None)
                )
                for i in range(4 * c, 4 * c + 4):
                    ps = [
                        psp.tile([128, 512], F32, tag="mm", bufs=4, name="psk")
                        for _ in range(2)
                    ]
                    first = None
                    for ct in range(8):
                        for nch in range(2):
                            mm = nc.tensor.matmul(
                                ps[nch][:, :],
                                w_sb[:, ct, i * 128 : (i + 1) * 128],
                                xo[:, ct, nch * 512 : (nch + 1) * 512],
                                start=(ct == 0),
                                stop=(ct == 7),
                            )
                            if first is None:
                                first = mm
                    adds = []
                    for nch in range(2):
                        out_ap = (
                            kh[:, i, nch * 512 : (nch + 1) * 512]
                            if which == "k"
                            else QT[:, i, nch * 512 : (nch + 1) * 512]
                        )
                        adds.append(
                            nc.vector.tensor_scalar_add(
                                out_ap, ps[nch][:, :], bqk_sb[:, boff + i : boff + i + 1]
                            )
                        )
                    pace_group(first, adds[-1])
                if which == "k":
                    nc.sync.dma_start(
                        k_in[c].rearrange("(i p) m -> p i m", p=128),
                        kh[:, 4 * c : 4 * c + 4, :],
                    )
                    nc.gpsimd.collective_compute(
                        "AllGather",
                        mybir.AluOpType.bypass,
                        replica_groups=groups,
                        ins=[k_in[c].opt()],
                        outs=[k_out[c].opt()],
                    )

            def v_tiles(c):
                # V for own token tiles j (all 16 heads); ones column at d=D
                # drives the softmax denominator in PV.
                for j in range(4 * c, 4 * c + 4):
                    ps = [
                        psp.tile([128, 8, D], F32, tag="mm", bufs=4, name="psv")
                        for _ in range(2)
                    ]
                    first = None
                    for ct in range(8):
                        for vch in range(2):
                            mm = nc.tensor.matmul(
                                ps[vch][:, :, :],
                                xo[:, ct, j * 128 : (j + 1) * 128],
                                wv[:, ct, vch * 512 : (vch + 1) * 512],
                                start=(ct == 0),
                                stop=(ct == 7),
                            )
                            if first is None:
                                first = mm
                    nc.vector.memset(vh[:, j, :, D : D + 1], 1.0)
                    adds = []
                    for vch in range(2):
                        adds.append(
                            nc.vector.tensor_tensor(
                                vh[:, j, vch * 8 : (vch + 1) * 8, 0:D],
                                ps[vch][:, :, :],
                                bvb[:, vch * 512 : (vch + 1) * 512].rearrange(
                                    "p (h e) -> p h e", e=D
                                ),
                                op=mybir.AluOpType.add,
                            )
                        )
                    pace_group(first, adds[-1])
                nc.sync.dma_start(
                    v_in[c].rearrange("(j p) f -> p j f", p=128),
                    vh[:, 4 * c : 4 * c + 4, :, :].rearrange("p j h e -> p j (h e)"),
                )
                nc.gpsimd.collective_compute(
                    "AllGather",
                    mybir.AluOpType.bypass,
                    replica_groups=groups,
                    ins=[v_in[c].opt()],
                    outs=[v_out[c].opt()],
                )

            # gather order = consumption order: K chunk0 at attention start,
            # V chunk0 ~8 iters in, V chunk1 ~8 iters later, K chunk1 only
            # from head 8 (~140us later).
            k_heads(0, "k")
            v_tiles(0)
            v_tiles(1)
            k_heads(1, "k")

            # ---- unstage gathered K chunk0 (scalar queue) ----
            for r in range(2):
                nc.scalar.dma_start(
                    KT[:, 0:4, r * NQ : (r + 1) * NQ],
                    k_out[0][r].rearrange("(i p) m -> p i m", p=128),
                )
            # V unstages ride the gpsimd queue (SWDGE)
            for c in range(2):
                for r in range(2):
                    nc.gpsimd.dma_start(
                        Vb[c][:, r, :, :, :].rearrange("p j h e -> p j (h e)"),
                        v_out[c][r].rearrange("(j p) f -> p j f", p=128),
                    )
            # K chunk1 unstage + proj weights on the sync queue (idle then)
            for r in range(2):
                nc.sync.dma_start(
                    KT[:, 4:8, r * NQ : (r + 1) * NQ],
                    k_out[1][r].rearrange("(i p) m -> p i m", p=128),
                )

            # ---- Q (own tokens; paced like K) ----
            k_heads(0, "q")
            k_heads(1, "q")

            lp.release()
            wk2 = tc.alloc_tile_pool(name="attnwork", bufs=1)
            wp_sb = wk2.tile([128, 8, C], BF16, tag="wp", name="wp")
            nc.sync.dma_start(
                wp_sb[:, :, :], wprojT.rearrange("(c p) o -> p c o", p=128)
            )

            # ---- attention ----
            # scores computed transposed (S^T[m, n]) in 512-col halves; PV
            # accumulates over all 16 m-tiles per head into one PSUM pair;
            # denominators = row D of the PV output via the ones-column of V.
            # Normalization of head h is deferred into head h+1's stream.
            pending = []

            def norm_a(ent):
                # stage PV out of PSUM, extract denominator, reciprocal
                h, pv = ent["h"], ent["pv"]
                stage = wk2.tile([65, NQ], BF16, tag="st", bufs=3, name="stage")
                den = wk2.tile([1, NQ], F32, tag="den", bufs=2, name="den")
                for nch in range(2):
                    nc.vector.tensor_copy(
                        stage[:, nch * 512 : (nch + 1) * 512], pv[nch][:, :]
                    )
                    nc.vector.tensor_copy(
                        den[:, nch * 512 : (nch + 1) * 512], pv[nch][64:65, :]
                    )
                rcp = wk2.tile([1, NQ], F32, tag="rcp", bufs=2, name="rcp")
                nc.vector.reciprocal_approx_fast(rcp[:, :], den[:, :])
                rb = wk2.tile([64, NQ], F32, tag="rb", bufs=2, name="rb")
                nc.gpsimd.partition_broadcast(rb[:, :], rcp[:, :])
                ent["stage"], ent["rb"] = stage, rb

            def norm_b(ent):
                h = ent["h"]
                i, poff = h // 2, (h % 2) * 64
                nc.vector.tensor_mul(
                    A_sb[i][poff : poff + 64, :], ent["stage"][0:64, :], ent["rb"][:, :]
                )

            for h in range(H):
                i, poff = h // 2, (h % 2) * 64
                pv = [
                    psp.tile([65, 512], F32, tag=f"acc{j}", bufs=2, name=f"pv{j}")
                    for j in range(2)
                ]
                for idx, mt in enumerate(MT_ORDER):
                    r, j = mt // 8, mt % 8
                    vc, vj = j // 4, j % 4
                    sp = [
                        psp.tile([128, 512], F32, tag="mm", bufs=4, name="pss")
                        for _ in range(2)
                    ]
                    p = wk2.tile([128, NQ], BF16, tag="p", bufs=6, name="p")
                    for nch in range(2):
                        nc.tensor.matmul(
                            sp[nch][:, :],
                            KT[poff : poff + 64, i, mt * 128 : (mt + 1) * 128],
                            QT[poff : poff + 64, i, nch * 512 : (nch + 1) * 512],
                            start=True,
                            stop=True,
                        )
                    for nch in range(2):
                        if idx in DVE_EXP_IDX:
                            nc.vector.tensor_scalar(
                                p[:, nch * 512 : (nch + 1) * 512].bitcast(I16),
                                sp[nch][:, :],
                                EXPA, EXPB,
                                op0=mybir.AluOpType.mult,
                                op1=mybir.AluOpType.add,
                            )
                        else:
                            nc.scalar.activation(
                                p[:, nch * 512 : (nch + 1) * 512], sp[nch][:, :],
                                mybir.ActivationFunctionType.Exp, scale=SCALE,
                            )
                    for nch in range(2):
                        nc.tensor.matmul(
                            pv[nch][:, :],
                            Vb[vc][:, r, vj, h, :],
                            p[:, nch * 512 : (nch + 1) * 512],
                            start=(idx == 0),
                            stop=(idx == 15),
                            skip_group_check=True,
                        )
                    if idx == 4 and pending:
                        norm_a(pending[0])
                    if idx == 9 and pending:
                        norm_b(pending.pop(0))
                pending.append({"h": h, "pv": pv})
            while pending:
                ent = pending.pop(0)
                norm_a(ent)
                norm_b(ent)

            # ---- output projection (ot pairs: 4 open accumulators) ----
            for op2 in range(4):
                pss = [
                    psp.tile([128, 512], F32, tag=f"acc{nch}", bufs=2, name="psp")
                    for j in range(2)
                    for nch in range(2)
                ]
                for dd in range(8):
                    for j in range(2):
                        ot = op2 * 2 + j
                        for nch in range(2):
                            nc.tensor.matmul(
                                pss[j * 2 + nch][:, :],
                                wp_sb[:, dd, ot * 128 : (ot + 1) * 128],
                                A_sb[dd][:, nch * 512 : (nch + 1) * 512],
                                start=(dd == 0),
                                stop=(dd == 7),
                            )
                for j in range(2):
                    ot = op2 * 2 + j
                    for nch in range(2):
                        y = wk2.tile([128, 512], F32, tag="y", bufs=3, name="y")
                        nc.vector.tensor_scalar_add(
                            y[:, :], pss[j * 2 + nch][:, :],
                            bp_sb[:, ot : ot + 1],
                        )
                        nc.scalar.dma_start(
                            yT[ot * 128 : (ot + 1) * 128, nch * 512 : (nch + 1) * 512],
                            y[:, :],
                        )
            wk2.release()

    nc.compile()
    return nc


def kernel(x, w_qkv, b_qkv, w_proj, b_proj):
    global LAST_RESULTS
    bf = ml_dtypes.bfloat16
    x = np.asarray(x, np.float32)
    w_qkv = np.asarray(w_qkv, np.float32)
    b_qkv = np.asarray(b_qkv, np.float32)
    w_proj = np.asarray(w_proj, np.float32)
    b_proj = np.asarray(b_proj, np.float32)

    wqkvT = np.ascontiguousarray(w_qkv.T.astype(bf))            # [1024, 3072]
    wprojT = np.ascontiguousarray(w_proj.T.astype(bf))          # [1024, 1024]
    bqk = np.ascontiguousarray(
        b_qkv[: 2 * C].reshape(16, 128).T.astype(np.float32)
    )                                                           # [128, 16]
    bv = np.ascontiguousarray(b_qkv[None, 2 * C :].astype(bf))  # [1, 1024]
    bproj = np.ascontiguousarray(
        b_proj.reshape(8, 128).T.astype(np.float32)
    )                                                           # [128, 8]

    in_maps = []
    for core in range(NCORES):
        b, half = core // 2, core % 2
        own = x[b][half * NQ : (half + 1) * NQ]                 # [1024, 1024]
        in_maps.append(
            {
                "xoT": np.ascontiguousarray(own.T.astype(bf)),
                "wqkvT": wqkvT,
                "bqk": bqk,
                "bv": bv,
                "wprojT": wprojT,
                "bproj": bproj,
            }
        )

    if "nc" not in _CACHE:
        _CACHE["nc"] = _build()
    nc = _CACHE["nc"]

    res = run_bass_kernel_spmd(nc, in_maps, core_ids=list(range(NCORES)))
    LAST_RESULTS = res

    out = np.empty((B, N, C), np.float32)
    for core in range(NCORES):
        b, half = core // 2, core % 2
        out[b, half * NQ : (half + 1) * NQ, :] = res.results[core]["yT"].T
    return out


if __name__ == "__main__":
    rng = np.random.default_rng(0)
    s = C ** -0.5
    ins = {
        "x": rng.standard_normal((B, N, C)).astype(np.float32),
        "w_qkv": (rng.standard_normal((3 * C, C)) * s).astype(np.float32),
        "b_qkv": (rng.standard_normal(3 * C) * 0.02).astype(np.float32),
        "w_proj": (rng.standard_normal((C, C)) * s).astype(np.float32),
        "b_proj": (rng.standard_normal(C) * 0.02).astype(np.float32),
    }
    y = kernel(**ins)
    print("out", y.shape, y.dtype, float(np.abs(y).mean()))


# revision 9
# speedup vs baseline: 1.3904x; 1.3904x over previous
"""Multi-head attention (B=4, N=2048, C=1024, H=16, D=64) on 8 TRN2 NeuronCores.

Sharding: core c owns (batch b = c//2, sequence half = c%2) -> 1024 query
tokens, all 16 heads.  Each core computes K and V for its OWN half only;
the partner half arrives via pairwise AllGathers (replica groups [2b, 2b+1]).
Output is purely row-sharded -> host gather is a concat.

Performance notes (v5):
- TRN2's activity-monitor firmware clamps the PE clock to 50% when PE
  activity stays near 100% for more than ~15-30 3.4us windows, and the clamp
  can persist for hundreds of us.  The QKV front is therefore PACED to ~65%
  activity (the attention phase's natural level, measured sustainable):
  each projection group's first matmul is gated on a small Vector-engine
  pace op chained behind the previous group's bias adds.  The front's wall
  time is bounded by the AllGather chain anyway, so pacing costs nothing.
- The four AllGathers are ordered by when their output is consumed
  (K chunk0, V chunk0, V chunk1, K chunk1) and attention iterates m-tiles
  grouped by V chunk, so no dependency is ever on the critical path.
- exp() is split between the Scalar engine (native Exp) and the Vector
  engine (Schraudolph bit-trick: bf16 is the top half of f32, so
  p = bitcast_bf16(int16(S*scale*184.665 + 16250.5)) is one tensor_scalar).
  Scores PSUM + exp are processed in 512-column halves (4 single-bank PSUM
  slots) so the PV matmuls wait on half-tiles, not full tiles.
- Each head's softmax normalization is deferred into the next head's
  iteration stream so the Vector queue never delays an exp.
- All matmuls bf16 with f32 PSUM accumulate.
"""

import numpy as np
import ml_dtypes

import concourse.bass as bass
import concourse.mybir as mybir
import concourse.tile as tile
from concourse import bacc
from concourse.bass import _add_dep_helper
from concourse.bass_utils import run_bass_kernel_spmd

B, N, C = 4, 2048, 1024
H, D = 16, 64
SCALE = D ** -0.5
NCORES = 8
NQ = N // 2          # query tokens per core (own half)
M = N                # key/value tokens after gather

BF16 = mybir.dt.bfloat16
F32 = mybir.dt.float32
I16 = mybir.dt.int16

# Schraudolph exp in bf16: exp(x*SCALE) ~= bitcast_bf16(int16(x*EXPA + EXPB))
EXPA = (2.0 ** 7 / np.log(2.0)) * SCALE
EXPB = 127.0 * 128.0 - 5.5
# which of the 16 m-tile iterations per head run exp on DVE instead of Scalar
DVE_EXP_IDX = {3, 7, 11, 14}
# m-tiles grouped by V gather chunk (j=mt%8: j<4 -> chunk0, j>=4 -> chunk1)
MT_ORDER = [0, 1, 2, 3, 8, 9, 10, 11, 4, 5, 6, 7, 12, 13, 14, 15]
PACE_N = 700         # pace-op length (f32 elems) -> ~0.5us on DVE

_CACHE = {}
LAST_RESULTS = None


def _build():
    nc = bacc.Bacc(
        "TRN2",
        target_bir_lowering=False,
        debug=False,
        enable_asserts=False,
        num_devices=NCORES,
    )
    xoT = nc.dram_tensor("xoT", [C, NQ], BF16, kind="ExternalInput")
    wqkvT = nc.dram_tensor("wqkvT", [C, 3 * C], BF16, kind="ExternalInput")
    bqk = nc.dram_tensor("bqk", [128, 16], F32, kind="ExternalInput")
    bv = nc.dram_tensor("bv", [1, C], BF16, kind="ExternalInput")
    wprojT = nc.dram_tensor("wprojT", [C, C], BF16, kind="ExternalInput")
    bproj = nc.dram_tensor("bproj", [128, 8], F32, kind="ExternalInput")
    yT = nc.dram_tensor("yT", [C, NQ], F32, kind="ExternalOutput")

    groups = [[2 * b, 2 * b + 1] for b in range(B)]

    with tile.TileContext(nc) as tc:
        with (
            tc.tile_pool(name="persist", bufs=1) as pp,
            tc.tile_pool(name="psum", bufs=1, space="PSUM") as psp,
            tc.tile_pool(name="dram", bufs=1, space="DRAM") as dp,
        ):
            lp = tc.alloc_tile_pool(name="front", bufs=1)

            # ---- inputs; wk on the scalar queue so K can start earliest ----
            wk = lp.tile([128, 8, C], BF16, tag="wk", name="wk")
            nc.scalar.dma_start(
                wk[:, :, :],
                wqkvT.rearrange("(c p) o -> p c o", p=128)[:, :, C : 2 * C],
            )
            xo = lp.tile([128, 8, NQ], BF16, tag="xo", name="xo")
            nc.sync.dma_start(xo[:, :, :], xoT.rearrange("(c p) n -> p c n", p=128))
            wv = lp.tile([128, 8, C], BF16, tag="wv", name="wv")
            nc.sync.dma_start(
                wv[:, :, :],
                wqkvT.rearrange("(c p) o -> p c o", p=128)[:, :, 2 * C : 3 * C],
            )
            wq = lp.tile([128, 8, C], BF16, tag="wq", name="wq")
            nc.sync.dma_start(
                wq[:, :, :],
                wqkvT.rearrange("(c p) o -> p c o", p=128)[:, :, 0:C],
            )

            bqk_sb = pp.tile([128, 16], F32, tag="bqk", name="bqk")
            nc.scalar.dma_start(bqk_sb[:, :], bqk[:, :])
            bv_sb = lp.tile([1, C], BF16, tag="bv", name="bv")
            nc.scalar.dma_start(bv_sb[:, :], bv[:, :])
            bp_sb = pp.tile([128, 8], F32, tag="bp", name="bp")
            nc.scalar.dma_start(bp_sb[:, :], bproj[:, :])

            bvb = lp.tile([128, C], BF16, tag="bvb", name="bvb")
            nc.gpsimd.partition_broadcast(bvb[:, :], bv_sb[:, :])

            # ---- persistent attention operands ----
            KT = pp.tile([128, 8, M], BF16, tag="KT", name="KT")
            QT = pp.tile([128, 8, NQ], BF16, tag="QT", name="QT")
            Vb = [
                pp.tile([128, 2, 4, H, D + 1], BF16, tag=f"Vb{c}", name=f"Vb{c}")
                for c in range(2)
            ]
            A_sb = [
                pp.tile([128, NQ], BF16, tag=f"a{i}", name=f"a{i}") for i in range(8)
            ]

            # staging SBUF + DRAM bounce buffers
            kh = lp.tile([128, 8, NQ], BF16, tag="kh", name="kh")
            vh = lp.tile([128, 8, H, D + 1], BF16, tag="vh", name="vh")
            k_in = [dp.tile([512, NQ], BF16, tag=f"ki{c}", name=f"ki{c}") for c in range(2)]
            k_out = [
                dp.tile([2, 512, NQ], BF16, tag=f"ko{c}", name=f"ko{c}") for c in range(2)
            ]
            v_in = [
                dp.tile([512, H * (D + 1)], BF16, tag=f"vi{c}", name=f"vi{c}")
                for c in range(2)
            ]
            v_out = [
                dp.tile([2, 512, H * (D + 1)], BF16, tag=f"vo{c}", name=f"vo{c}")
                for c in range(2)
            ]

            # ---- HAM pacing machinery ----
            pace_sb = lp.tile([1, 2 * PACE_N], F32, tag="pace", name="pace")
            nc.vector.memset(pace_sb[:, :], 0.0)
            pace_state = {"last": None, "flip": 0}

            def pace_group(first_mm, last_dve):
                # gate this group's first matmul on the previous group's pace
                # op; chain a new pace op behind this group's bias adds.
                if pace_state["last"] is not None:
                    _add_dep_helper(
                        first_mm.ins, pace_state["last"].ins, sync=True,
                        reason="HAM activity pacing",
                    )
                f = pace_state["flip"]
                pace_state["flip"] = 1 - f
                pace_state["last"] = nc.vector.tensor_copy(
                    pace_sb[:, f * PACE_N : (f + 1) * PACE_N],
                    pace_sb[:, (1 - f) * PACE_N : (2 - f) * PACE_N],
                )
                # anchor the pace op behind this group's bias adds so the
                # scheduler cannot hoist the pace chain to the start
                _add_dep_helper(
                    pace_state["last"].ins, last_dve.ins, sync=False,
                    reason="HAM pacing anchor",
                )

            def k_heads(c, which):
                # K/Q output channels i*128..(i+1)*128 for own tokens; bias
                # fused into the PSUM->SBUF copy (split in halves for pacing).
                w_sb, boff, dst = (
                    (wk, 8, kh) if which == "k" else (wq, 0, None)
                )
                for i in range(4 * c, 4 * c + 4):
                    ps = [
                        psp.tile([128, 512], F32, tag="mm", bufs=4, name="psk")
                        for _ in range(2)
                    ]
                    first = None
                    for ct in range(8):
                        for nch in range(2):
                            mm = nc.tensor.matmul(
                                ps[nch][:, :],
                                w_sb[:, ct, i * 128 : (i + 1) * 128],
                                xo[:, ct, nch * 512 : (nch + 1) * 512],
                                start=(ct == 0),
                                stop=(ct == 7),
                            )
                            if first is None:
                                first = mm
                    adds = []
                    for nch in range(2):
                        out_ap = (
                            kh[:, i, nch * 512 : (nch + 1) * 512]
                            if which == "k"
                            else QT[:, i, nch * 512 : (nch + 1) * 512]
                        )
                        adds.append(
                            nc.vector.tensor_scalar_add(
                                out_ap, ps[nch][:, :], bqk_sb[:, boff + i : boff + i + 1]
                            )
                        )
                    pace_group(first, adds[-1])
                if which == "k":
                    nc.sync.dma_start(
                        k_in[c].rearrange("(i p) m -> p i m", p=128),
                        kh[:, 4 * c : 4 * c + 4, :],
                    )
                    nc.gpsimd.collective_compute(
                        "AllGather",
                        mybir.AluOpType.bypass,
                        replica_groups=groups,
                        ins=[k_in[c].opt()],
                        outs=[k_out[c].opt()],
                    )

            def v_tiles(c):
                # V for own token tiles j (all 16 heads); ones column at d=D
                # drives the softmax denominator in PV.
                for j in range(4 * c, 4 * c + 4):
                    ps = [
                        psp.tile([128, 8, D], F32, tag="mm", bufs=4, name="psv")
                        for _ in range(2)
                    ]
                    first = None
                    for ct in range(8):
                        for vch in range(2):
                            mm = nc.tensor.matmul(
                                ps[vch][:, :, :],
                                xo[:, ct, j * 128 : (j + 1) * 128],
                                wv[:, ct, vch * 512 : (vch + 1) * 512],
                                start=(ct == 0),
                                stop=(ct == 7),
                            )
                            if first is None:
                                first = mm
                    nc.vector.memset(vh[:, j, :, D : D + 1], 1.0)
                    adds = []
                    for vch in range(2):
                        adds.append(
                            nc.vector.tensor_tensor(
                                vh[:, j, vch * 8 : (vch + 1) * 8, 0:D],
                                ps[vch][:, :, :],
                                bvb[:, vch * 512 : (vch + 1) * 512].rearrange(
                                    "p (h e) -> p h e", e=D
                                ),
                                op=mybir.AluOpType.add,
                            )
                        )
                    pace_group(first, adds[-1])
                nc.sync.dma_start(
                    v_in[c].rearrange("(j p) f -> p j f", p=128),
                    vh[:, 4 * c : 4 * c + 4, :, :].rearrange("p j h e -> p j (h e)"),
                )
                nc.gpsimd.collective_compute(
                    "AllGather",
                    mybir.AluOpType.bypass,
                    replica_groups=groups,
                    ins=[v_in[c].opt()],
                    outs=[v_out[c].opt()],
                )

            # gather order = consumption order: K chunk0 at attention start,
            # V chunk0 ~8 iters in, V chunk1 ~8 iters later, K chunk1 only
            # from head 8 (~140us later).
            k_heads(0, "k")
            v_tiles(0)
            v_tiles(1)
            k_heads(1, "k")

            # ---- unstage gathered K chunk0 (scalar queue) ----
            for r in range(2):
                nc.scalar.dma_start(
                    KT[:, 0:4, r * NQ : (r + 1) * NQ],
                    k_out[0][r].rearrange("(i p) m -> p i m", p=128),
                )
            # V unstages ride the gpsimd queue (SWDGE)
            for c in range(2):
                for r in range(2):
                    nc.gpsimd.dma_start(
                        Vb[c][:, r, :, :, :].rearrange("p j h e -> p j (h e)"),
                        v_out[c][r].rearrange("(j p) f -> p j f", p=128),
                    )
            # K chunk1 unstage + proj weights on the sync queue (idle then)
            for r in range(2):
                nc.sync.dma_start(
                    KT[:, 4:8, r * NQ : (r + 1) * NQ],
                    k_out[1][r].rearrange("(i p) m -> p i m", p=128),
                )

            # ---- Q (own tokens; paced like K) ----
            k_heads(0, "q")
            k_heads(1, "q")

            lp.release()
            wk2 = tc.alloc_tile_pool(name="attnwork", bufs=1)
            wp_sb = wk2.tile([128, 8, C], BF16, tag="wp", name="wp")
            nc.sync.dma_start(
                wp_sb[:, :, :], wprojT.rearrange("(c p) o -> p c o", p=128)
            )

            # ---- attention ----
            # scores computed transposed (S^T[m, n]) in 512-col halves; PV
            # accumulates over all 16 m-tiles per head into one PSUM pair;
            # denominators = row D of the PV output via the ones-column of V.
            # Normalization of head h is deferred into head h+1's stream.
            pending = []

            def norm_a(ent):
                # stage PV out of PSUM, extract denominator, reciprocal
                h, pv = ent["h"], ent["pv"]
                stage = wk2.tile([65, NQ], BF16, tag="st", bufs=3, name="stage")
                den = wk2.tile([1, NQ], F32, tag="den", bufs=2, name="den")
                for nch in range(2):
                    nc.vector.tensor_copy(
                        stage[:, nch * 512 : (nch + 1) * 512], pv[nch][:, :]
                    )
                    nc.vector.tensor_copy(
                        den[:, nch * 512 : (nch + 1) * 512], pv[nch][64:65, :]
                    )
                rcp = wk2.tile([1, NQ], F32, tag="rcp", bufs=2, name="rcp")
                nc.vector.reciprocal_approx_fast(rcp[:, :], den[:, :])
                rb = wk2.tile([64, NQ], F32, tag="rb", bufs=2, name="rb")
                nc.gpsimd.partition_broadcast(rb[:, :], rcp[:, :])
                ent["stage"], ent["rb"] = stage, rb

            def norm_b(ent):
                h = ent["h"]
                i, poff = h // 2, (h % 2) * 64
                nc.vector.tensor_mul(
                    A_sb[i][poff : poff + 64, :], ent["stage"][0:64, :], ent["rb"][:, :]
                )

            for h in range(H):
                i, poff = h // 2, (h % 2) * 64
                pv = [
                    psp.tile([65, 512], F32, tag=f"acc{j}", bufs=2, name=f"pv{j}")
                    for j in range(2)
                ]
                for idx, mt in enumerate(MT_ORDER):
                    r, j = mt // 8, mt % 8
                    vc, vj = j // 4, j % 4
                    sp = [
                        psp.tile([128, 512], F32, tag="mm", bufs=4, name="pss")
                        for _ in range(2)
                    ]
                    p = wk2.tile([128, NQ], BF16, tag="p", bufs=6, name="p")
                    for nch in range(2):
                        nc.tensor.matmul(
                            sp[nch][:, :],
                            KT[poff : poff + 64, i, mt * 128 : (mt + 1) * 128],
                            QT[poff : poff + 64, i, nch * 512 : (nch + 1) * 512],
                            start=True,
                            stop=True,
                        )
                    for nch in range(2):
                        if idx in DVE_EXP_IDX:
                            nc.vector.tensor_scalar(
                                p[:, nch * 512 : (nch + 1) * 512].bitcast(I16),
                                sp[nch][:, :],
                                EXPA, EXPB,
                                op0=mybir.AluOpType.mult,
                                op1=mybir.AluOpType.add,
                            )
                        else:
                            nc.scalar.activation(
                                p[:, nch * 512 : (nch + 1) * 512], sp[nch][:, :],
                                mybir.ActivationFunctionType.Exp, scale=SCALE,
                            )
                    for nch in range(2):
                        nc.tensor.matmul(
                            pv[nch][:, :],
                            Vb[vc][:, r, vj, h, :],
                            p[:, nch * 512 : (nch + 1) * 512],
                            start=(idx == 0),
                            stop=(idx == 15),
                            skip_group_check=True,
                        )
                    if idx == 4 and pending:
                        norm_a(pending[0])
                    if idx == 9 and pending:
                        norm_b(pending.pop(0))
                pending.append({"h": h, "pv": pv})
            while pending:
                ent = pending.pop(0)
                norm_a(ent)
                norm_b(ent)

            # ---- output projection (ot pairs: 4 open accumulators) ----
            for op2 in range(4):
                pss = [
                    psp.tile([128, 512], F32, tag=f"acc{nch}", bufs=2, name="psp")
                    for j in range(2)
                    for nch in range(2)
                ]
                for dd in range(8):
                    for j in range(2):
                        ot = op2 * 2 + j
                        for nch in range(2):
                            nc.tensor.matmul(
                                pss[j * 2 + nch][:, :],
                                wp_sb[:, dd, ot * 128 : (ot + 1) * 128],
                                A_sb[dd][:, nch * 512 : (nch + 1) * 512],
                                start=(dd == 0),
                                stop=(dd == 7),
                            )
                for j in range(2):
                    ot = op2 * 2 + j
                    for nch in range(2):
                        y = wk2.tile([128, 512], F32, tag="y", bufs=3, name="y")
                        nc.vector.tensor_scalar_add(
                            y[:, :], pss[j * 2 + nch][:, :],
                            bp_sb[:, ot : ot + 1],
                        )
                        nc.scalar.dma_start(
                            yT[ot * 128 : (ot + 1) * 128, nch * 512 : (nch + 1) * 512],
                            y[:, :],
                        )
            wk2.release()

    nc.compile()
    return nc


def kernel(x, w_qkv, b_qkv, w_proj, b_proj):
    global LAST_RESULTS
    bf = ml_dtypes.bfloat16
    x = np.asarray(x, np.float32)
    w_qkv = np.asarray(w_qkv, np.float32)
    b_qkv = np.asarray(b_qkv, np.float32)
    w_proj = np.asarray(w_proj, np.float32)
    b_proj = np.asarray(b_proj, np.float32)

    wqkvT = np.ascontiguousarray(w_qkv.T.astype(bf))            # [1024, 3072]
    wprojT = np.ascontiguousarray(w_proj.T.astype(bf))          # [1024, 1024]
    bqk = np.ascontiguousarray(
        b_qkv[: 2 * C].reshape(16, 128).T.astype(np.float32)
    )                                                           # [128, 16]
    bv = np.ascontiguousarray(b_qkv[None, 2 * C :].astype(bf))  # [1, 1024]
    bproj = np.ascontiguousarray(
        b_proj.reshape(8, 128).T.astype(np.float32)
    )                                                           # [128, 8]

    in_maps = []
    for core in range(NCORES):
        b, half = core // 2, core % 2
        own = x[b][half * NQ : (half + 1) * NQ]                 # [1024, 1024]
        in_maps.append(
            {
                "xoT": np.ascontiguousarray(own.T.astype(bf)),
                "wqkvT": wqkvT,
                "bqk": bqk,
                "bv": bv,
                "wprojT": wprojT,
                "bproj": bproj,
            }
        )

    if "nc" not in _CACHE:
        _CACHE["nc"] = _build()
    nc = _CACHE["nc"]

    res = run_bass_kernel_spmd(nc, in_maps, core_ids=list(range(NCORES)))
    LAST_RESULTS = res

    out = np.empty((B, N, C), np.float32)
    for core in range(NCORES):
        b, half = core // 2, core % 2
        out[b, half * NQ : (half + 1) * NQ, :] = res.results[core]["yT"].T
    return out


if __name__ == "__main__":
    rng = np.random.default_rng(0)
    s = C ** -0.5
    ins = {
        "x": rng.standard_normal((B, N, C)).astype(np.float32),
        "w_qkv": (rng.standard_normal((3 * C, C)) * s).astype(np.float32),
        "b_qkv": (rng.standard_normal(3 * C) * 0.02).astype(np.float32),
        "w_proj": (rng.standard_normal((C, C)) * s).astype(np.float32),
        "b_proj": (rng.standard_normal(C) * 0.02).astype(np.float32),
    }
    y = kernel(**ins)
    print("out", y.shape, y.dtype, float(np.abs(y).mean()))


# revision 10
# speedup vs baseline: 1.4139x; 1.0169x over previous
"""Multi-head attention (B=4, N=2048, C=1024, H=16, D=64) on 8 TRN2 NeuronCores.

Sharding: core c owns (batch b = c//2, sequence half = c%2) -> 1024 query
tokens, all 16 heads.  Each core computes K and V for its OWN half only;
the partner half arrives via pairwise AllGathers (replica groups [2b, 2b+1]).
Output is purely row-sharded -> host gather is a concat.

Performance notes (v5):
- TRN2's activity-monitor firmware clamps the PE clock to 50% when PE
  activity stays near 100% for more than ~15-30 3.4us windows, and the clamp
  can persist for hundreds of us.  The QKV front is therefore PACED to ~65%
  activity (the attention phase's natural level, measured sustainable):
  each projection group's first matmul is gated on a small Vector-engine
  pace op chained behind the previous group's bias adds.  The front's wall
  time is bounded by the AllGather chain anyway, so pacing costs nothing.
- The four AllGathers are ordered by when their output is consumed
  (K chunk0, V chunk0, V chunk1, K chunk1) and attention iterates m-tiles
  grouped by V chunk, so no dependency is ever on the critical path.
- exp() is split between the Scalar engine (native Exp) and the Vector
  engine (Schraudolph bit-trick: bf16 is the top half of f32, so
  p = bitcast_bf16(int16(S*scale*184.665 + 16250.5)) is one tensor_scalar).
  Scores PSUM + exp are processed in 512-column halves (4 single-bank PSUM
  slots) so the PV matmuls wait on half-tiles, not full tiles.
- Each head's softmax normalization is deferred into the next head's
  iteration stream so the Vector queue never delays an exp.
- All matmuls bf16 with f32 PSUM accumulate.
"""

import numpy as np
import ml_dtypes

import concourse.bass as bass
import concourse.mybir as mybir
import concourse.tile as tile
from concourse import bacc
from concourse.bass import _add_dep_helper
from concourse.bass_utils import run_bass_kernel_spmd

B, N, C = 4, 2048, 1024
H, D = 16, 64
SCALE = D ** -0.5
NCORES = 8
NQ = N // 2          # query tokens per core (own half)
M = N                # key/value tokens after gather

BF16 = mybir.dt.bfloat16
F32 = mybir.dt.float32
I16 = mybir.dt.int16

# Schraudolph exp in bf16: exp(x*SCALE) ~= bitcast_bf16(int16(x*EXPA + EXPB))
EXPA = (2.0 ** 7 / np.log(2.0)) * SCALE
EXPB = 127.0 * 128.0 - 5.5
# which of the 16 m-tile iterations per head run exp on DVE instead of Scalar
DVE_EXP_IDX = {3, 7, 11, 14}
# m-tiles grouped by V gather chunk (j=mt%8: j<4 -> chunk0, j>=4 -> chunk1)
MT_ORDER = [0, 1, 2, 3, 8, 9, 10, 11, 4, 5, 6, 7, 12, 13, 14, 15]
PACE_N = 700         # pace-op length (f32 elems) -> ~0.5us on DVE

_CACHE = {}
LAST_RESULTS = None


def _build():
    nc = bacc.Bacc(
        "TRN2",
        target_bir_lowering=False,
        debug=False,
        enable_asserts=False,
        num_devices=NCORES,
    )
    xoT = nc.dram_tensor("xoT", [C, NQ], BF16, kind="ExternalInput")
    wqkvT = nc.dram_tensor("wqkvT", [C, 3 * C], BF16, kind="ExternalInput")
    bqk = nc.dram_tensor("bqk", [128, 16], F32, kind="ExternalInput")
    bv = nc.dram_tensor("bv", [1, C], BF16, kind="ExternalInput")
    wprojT = nc.dram_tensor("wprojT", [C, C], BF16, kind="ExternalInput")
    bproj = nc.dram_tensor("bproj", [128, 8], F32, kind="ExternalInput")
    yT = nc.dram_tensor("yT", [C, NQ], F32, kind="ExternalOutput")

    groups = [[2 * b, 2 * b + 1] for b in range(B)]

    with tile.TileContext(nc) as tc:
        with (
            tc.tile_pool(name="persist", bufs=1) as pp,
            tc.tile_pool(name="psum", bufs=1, space="PSUM") as psp,
            tc.tile_pool(name="dram", bufs=1, space="DRAM") as dp,
        ):
            lp = tc.alloc_tile_pool(name="front", bufs=1)

            # ---- inputs; wk on the scalar queue so K can start earliest ----
            wk = lp.tile([128, 8, C], BF16, tag="wk", name="wk")
            nc.scalar.dma_start(
                wk[:, :, :],
                wqkvT.rearrange("(c p) o -> p c o", p=128)[:, :, C : 2 * C],
            )
            xo = lp.tile([128, 8, NQ], BF16, tag="xo", name="xo")
            nc.sync.dma_start(xo[:, :, :], xoT.rearrange("(c p) n -> p c n", p=128))
            wv = lp.tile([128, 8, C], BF16, tag="wv", name="wv")
            nc.sync.dma_start(
                wv[:, :, :],
                wqkvT.rearrange("(c p) o -> p c o", p=128)[:, :, 2 * C : 3 * C],
            )
            wq = lp.tile([128, 8, C], BF16, tag="wq", name="wq")
            nc.sync.dma_start(
                wq[:, :, :],
                wqkvT.rearrange("(c p) o -> p c o", p=128)[:, :, 0:C],
            )

            bqk_sb = pp.tile([128, 16], F32, tag="bqk", name="bqk")
            nc.scalar.dma_start(bqk_sb[:, :], bqk[:, :])
            bv_sb = lp.tile([1, C], BF16, tag="bv", name="bv")
            nc.scalar.dma_start(bv_sb[:, :], bv[:, :])
            bp_sb = pp.tile([128, 8], F32, tag="bp", name="bp")
            nc.scalar.dma_start(bp_sb[:, :], bproj[:, :])

            bvb = lp.tile([128, C], BF16, tag="bvb", name="bvb")
            nc.gpsimd.partition_broadcast(bvb[:, :], bv_sb[:, :])

            # ---- persistent attention operands ----
            KT = pp.tile([128, 8, M], BF16, tag="KT", name="KT")
            QT = pp.tile([128, 8, NQ], BF16, tag="QT", name="QT")
            Vb = [
                pp.tile([128, 2, 4, H, D + 1], BF16, tag=f"Vb{c}", name=f"Vb{c}")
                for c in range(2)
            ]
            A_sb = [
                pp.tile([128, NQ], BF16, tag=f"a{i}", name=f"a{i}") for i in range(8)
            ]

            # staging SBUF + DRAM bounce buffers
            kh = lp.tile([128, 8, NQ], BF16, tag="kh", name="kh")
            vh = lp.tile([128, 8, H, D + 1], BF16, tag="vh", name="vh")
            k_in = [dp.tile([512, NQ], BF16, tag=f"ki{c}", name=f"ki{c}") for c in range(2)]
            k_out = [
                dp.tile([2, 512, NQ], BF16, tag=f"ko{c}", name=f"ko{c}") for c in range(2)
            ]
            v_in = [
                dp.tile([512, H * (D + 1)], BF16, tag=f"vi{c}", name=f"vi{c}")
                for c in range(2)
            ]
            v_out = [
                dp.tile([2, 512, H * (D + 1)], BF16, tag=f"vo{c}", name=f"vo{c}")
                for c in range(2)
            ]

            # ---- HAM pacing machinery ----
            pace_sb = lp.tile([1, 2 * PACE_N], F32, tag="pace", name="pace")
            nc.vector.memset(pace_sb[:, :], 0.0)
            pace_state = {"last": None, "flip": 0}

            def pace_group(first_mm, last_dve):
                # gate this group's first matmul on the previous group's pace
                # op; chain a new pace op behind this group's bias adds.
                if pace_state["last"] is not None:
                    _add_dep_helper(
                        first_mm.ins, pace_state["last"].ins, sync=True,
                        reason="HAM activity pacing",
                    )
                f = pace_state["flip"]
                pace_state["flip"] = 1 - f
                pace_state["last"] = nc.vector.tensor_copy(
                    pace_sb[:, f * PACE_N : (f + 1) * PACE_N],
                    pace_sb[:, (1 - f) * PACE_N : (2 - f) * PACE_N],
                )
                # anchor the pace op behind this group's bias adds so the
                # scheduler cannot hoist the pace chain to the start
                _add_dep_helper(
                    pace_state["last"].ins, last_dve.ins, sync=False,
                    reason="HAM pacing anchor",
                )

            def k_heads(c, which):
                # K/Q output channels i*128..(i+1)*128 for own tokens; bias
                # fused into the PSUM->SBUF copy (split in halves for pacing).
                w_sb, boff, dst = (
                    (wk, 8, kh) if which == "k" else (wq, 0, None)
                )
                for i in range(4 * c, 4 * c + 4):
                    ps = [
                        psp.tile([128, 512], F32, tag="mm", bufs=4, name="psk")
                        for _ in range(2)
                    ]
                    first = None
                    for ct in range(8):
                        for nch in range(2):
                            mm = nc.tensor.matmul(
                                ps[nch][:, :],
                                w_sb[:, ct, i * 128 : (i + 1) * 128],
                                xo[:, ct, nch * 512 : (nch + 1) * 512],
                                start=(ct == 0),
                                stop=(ct == 7),
                            )
                            if first is None:
                                first = mm
                    adds = []
                    for nch in range(2):
                        out_ap = (
                            kh[:, i, nch * 512 : (nch + 1) * 512]
                            if which == "k"
                            else QT[:, i, nch * 512 : (nch + 1) * 512]
                        )
                        adds.append(
                            nc.vector.tensor_scalar_add(
                                out_ap, ps[nch][:, :], bqk_sb[:, boff + i : boff + i + 1]
                            )
                        )
                    pace_group(first, adds[-1])
                if which == "k":
                    nc.sync.dma_start(
                        k_in[c].rearrange("(i p) m -> p i m", p=128),
                        kh[:, 4 * c : 4 * c + 4, :],
                    )
                    nc.gpsimd.collective_compute(
                        "AllGather",
                        mybir.AluOpType.bypass,
                        replica_groups=groups,
                        ins=[k_in[c].opt()],
                        outs=[k_out[c].opt()],
                    )

            def v_tiles(c):
                # V for own token tiles j (all 16 heads); ones column at d=D
                # drives the softmax denominator in PV.
                for j in range(4 * c, 4 * c + 4):
                    ps = [
                        psp.tile([128, 8, D], F32, tag="mm", bufs=4, name="psv")
                        for _ in range(2)
                    ]
                    first = None
                    for ct in range(8):
                        for vch in range(2):
                            mm = nc.tensor.matmul(
                                ps[vch][:, :, :],
                                xo[:, ct, j * 128 : (j + 1) * 128],
                                wv[:, ct, vch * 512 : (vch + 1) * 512],
                                start=(ct == 0),
                                stop=(ct == 7),
                            )
                            if first is None:
                                first = mm
                    nc.vector.memset(vh[:, j, :, D : D + 1], 1.0)
                    adds = []
                    for vch in range(2):
                        adds.append(
                            nc.vector.tensor_tensor(
                                vh[:, j, vch * 8 : (vch + 1) * 8, 0:D],
                                ps[vch][:, :, :],
                                bvb[:, vch * 512 : (vch + 1) * 512].rearrange(
                                    "p (h e) -> p h e", e=D
                                ),
                                op=mybir.AluOpType.add,
                            )
                        )
                    pace_group(first, adds[-1])
                nc.sync.dma_start(
                    v_in[c].rearrange("(j p) f -> p j f", p=128),
                    vh[:, 4 * c : 4 * c + 4, :, :].rearrange("p j h e -> p j (h e)"),
                )
                nc.gpsimd.collective_compute(
                    "AllGather",
                    mybir.AluOpType.bypass,
                    replica_groups=groups,
                    ins=[v_in[c].opt()],
                    outs=[v_out[c].opt()],
                )

            # gather order = consumption order: K chunk0 at attention start,
            # V chunk0 ~8 iters in, V chunk1 ~8 iters later, K chunk1 only
            # from head 8 (~140us later).
            k_heads(0, "k")
            v_tiles(0)
            v_tiles(1)
            k_heads(1, "k")

            # ---- unstage gathered K chunk0 (scalar queue) ----
            for r in range(2):
                nc.scalar.dma_start(
                    KT[:, 0:4, r * NQ : (r + 1) * NQ],
                    k_out[0][r].rearrange("(i p) m -> p i m", p=128),
                )
            # V unstages ride the gpsimd queue (SWDGE)
            for c in range(2):
                for r in range(2):
                    nc.gpsimd.dma_start(
                        Vb[c][:, r, :, :, :].rearrange("p j h e -> p j (h e)"),
                        v_out[c][r].rearrange("(j p) f -> p j f", p=128),
                    )
            # K chunk1 unstage + proj weights on the sync queue (idle then)
            for r in range(2):
                nc.sync.dma_start(
                    KT[:, 4:8, r * NQ : (r + 1) * NQ],
                    k_out[1][r].rearrange("(i p) m -> p i m", p=128),
                )

            # ---- Q (own tokens; paced like K) ----
            k_heads(0, "q")
            k_heads(1, "q")

            lp.release()
            wk2 = tc.alloc_tile_pool(name="attnwork", bufs=1)
            wp_sb = wk2.tile([128, 8, C], BF16, tag="wp", name="wp")
            nc.sync.dma_start(
                wp_sb[:, :, :], wprojT.rearrange("(c p) o -> p c o", p=128)
            )

            # ---- attention ----
            # scores computed transposed (S^T[m, n]) in 512-col halves; PV
            # accumulates over all 16 m-tiles per head into one PSUM pair;
            # denominators = row D of the PV output via the ones-column of V.
            # Normalization of head h is deferred into head h+1's stream.
            pending = []

            def norm_a(ent):
                # stage PV out of PSUM, extract denominator, reciprocal
                h, pv = ent["h"], ent["pv"]
                stage = wk2.tile([65, NQ], BF16, tag="st", bufs=3, name="stage")
                den = wk2.tile([1, NQ], F32, tag="den", bufs=2, name="den")
                for nch in range(2):
                    nc.vector.tensor_copy(
                        stage[:, nch * 512 : (nch + 1) * 512], pv[nch][:, :]
                    )
                    nc.vector.tensor_copy(
                        den[:, nch * 512 : (nch + 1) * 512], pv[nch][64:65, :]
                    )
                rcp = wk2.tile([1, NQ], F32, tag="rcp", bufs=2, name="rcp")
                nc.vector.reciprocal_approx_fast(rcp[:, :], den[:, :])
                rb = wk2.tile([64, NQ], F32, tag="rb", bufs=2, name="rb")
                nc.gpsimd.partition_broadcast(rb[:, :], rcp[:, :])
                ent["stage"], ent["rb"] = stage, rb

            def norm_b(ent):
                h = ent["h"]
                i, poff = h // 2, (h % 2) * 64
                nc.vector.tensor_mul(
                    A_sb[i][poff : poff + 64, :], ent["stage"][0:64, :], ent["rb"][:, :]
                )

            for h in range(H):
                i, poff = h // 2, (h % 2) * 64
                pv = [
                    psp.tile([65, 512], F32, tag=f"acc{j}", bufs=2, name=f"pv{j}")
                    for j in range(2)
                ]

                def emit_pv(ent):
                    # PV runs one iteration behind scores/exp so the PE never
                    # waits on the exp chain (software pipelining; the Tensor
                    # queue executes in program order).
                    mt, idx, p = ent
                    r, j = mt // 8, mt % 8
                    vc, vj = j // 4, j % 4
                    for nch in range(2):
                        nc.tensor.matmul(
                            pv[nch][:, :],
                            Vb[vc][:, r, vj, h, :],
                            p[:, nch * 512 : (nch + 1) * 512],
                            start=(idx == 0),
                            stop=(idx == 15),
                            skip_group_check=True,
                        )

                prev = None
                for idx, mt in enumerate(MT_ORDER):
                    sp = [
                        psp.tile([128, 512], F32, tag="mm", bufs=4, name="pss")
                        for _ in range(2)
                    ]
                    p = wk2.tile([128, NQ], BF16, tag="p", bufs=6, name="p")
                    for nch in range(2):
                        nc.tensor.matmul(
                            sp[nch][:, :],
                            KT[poff : poff + 64, i, mt * 128 : (mt + 1) * 128],
                            QT[poff : poff + 64, i, nch * 512 : (nch + 1) * 512],
                            start=True,
                            stop=True,
                        )
                    for nch in range(2):
                        if idx in DVE_EXP_IDX:
                            nc.vector.tensor_scalar(
                                p[:, nch * 512 : (nch + 1) * 512].bitcast(I16),
                                sp[nch][:, :],
                                EXPA, EXPB,
                                op0=mybir.AluOpType.mult,
                                op1=mybir.AluOpType.add,
                            )
                        else:
                            nc.scalar.activation(
                                p[:, nch * 512 : (nch + 1) * 512], sp[nch][:, :],
                                mybir.ActivationFunctionType.Exp, scale=SCALE,
                            )
                    if prev is not None:
                        emit_pv(prev)
                    prev = (mt, idx, p)
                    if idx == 4 and pending:
                        norm_a(pending[0])
                    if idx == 9 and pending:
                        norm_b(pending.pop(0))
                emit_pv(prev)
                pending.append({"h": h, "pv": pv})
            while pending:
                ent = pending.pop(0)
                norm_a(ent)
                norm_b(ent)

            # ---- output projection (ot pairs: 4 open accumulators) ----
            for op2 in range(4):
                pss = [
                    psp.tile([128, 512], F32, tag=f"acc{nch}", bufs=2, name="psp")
                    for j in range(2)
                    for nch in range(2)
                ]
                for dd in range(8):
                    for j in range(2):
                        ot = op2 * 2 + j
                        for nch in range(2):
                            nc.tensor.matmul(
                                pss[j * 2 + nch][:, :],
                                wp_sb[:, dd, ot * 128 : (ot + 1) * 128],
                                A_sb[dd][:, nch * 512 : (nch + 1) * 512],
                                start=(dd == 0),
                                stop=(dd == 7),
                            )
                for j in range(2):
                    ot = op2 * 2 + j
                    for nch in range(2):
                        y = wk2.tile([128, 512], F32, tag="y", bufs=3, name="y")
                        nc.vector.tensor_scalar_add(
                            y[:, :], pss[j * 2 + nch][:, :],
                            bp_sb[:, ot : ot + 1],
                        )
                        nc.scalar.dma_start(
                            yT[ot * 128 : (ot + 1) * 128, nch * 512 : (nch + 1) * 512],
                            y[:, :],
                        )
            wk2.release()

    nc.compile()
    return nc


def kernel(x, w_qkv, b_qkv, w_proj, b_proj):
    global LAST_RESULTS
    bf = ml_dtypes.bfloat16
    x = np.asarray(x, np.float32)
    w_qkv = np.asarray(w_qkv, np.float32)
    b_qkv = np.asarray(b_qkv, np.float32)
    w_proj = np.asarray(w_proj, np.float32)
    b_proj = np.asarray(b_proj, np.float32)

    wqkvT = np.ascontiguousarray(w_qkv.T.astype(bf))            # [1024, 3072]
    wprojT = np.ascontiguousarray(w_proj.T.astype(bf))          # [1024, 1024]
    bqk = np.ascontiguousarray(
        b_qkv[: 2 * C].reshape(16, 128).T.astype(np.float32)
    )                                                           # [128, 16]
    bv = np.ascontiguousarray(b_qkv[None, 2 * C :].astype(bf))  # [1, 1024]
    bproj = np.ascontiguousarray(
        b_proj.reshape(8, 128).T.astype(np.float32)
    )                                                           # [128, 8]

    in_maps = []
    for core in range(NCORES):
        b, half = core // 2, core % 2
        own = x[b][half * NQ : (half + 1) * NQ]                 # [1024, 1024]
        in_maps.append(
            {
                "xoT": np.ascontiguousarray(own.T.astype(bf)),
                "wqkvT": wqkvT,
                "bqk": bqk,
                "bv": bv,
                "wprojT": wprojT,
                "bproj": bproj,
            }
        )

    if "nc" not in _CACHE:
        _CACHE["nc"] = _build()
    nc = _CACHE["nc"]

    res = run_bass_kernel_spmd(nc, in_maps, core_ids=list(range(NCORES)))
    LAST_RESULTS = res

    out = np.empty((B, N, C), np.float32)
    for core in range(NCORES):
        b, half = core // 2, core % 2
        out[b, half * NQ : (half + 1) * NQ, :] = res.results[core]["yT"].T
    return out


if __name__ == "__main__":
    rng = np.random.default_rng(0)
    s = C ** -0.5
    ins = {
        "x": rng.standard_normal((B, N, C)).astype(np.float32),
        "w_qkv": (rng.standard_normal((3 * C, C)) * s).astype(np.float32),
        "b_qkv": (rng.standard_normal(3 * C) * 0.02).astype(np.float32),
        "w_proj": (rng.standard_normal((C, C)) * s).astype(np.float32),
        "b_proj": (rng.standard_normal(C) * 0.02).astype(np.float32),
    }
    y = kernel(**ins)
    print("out", y.shape, y.dtype, float(np.abs(y).mean()))
